# revision 2
# baseline (speedup 1.0000x reference)
"""MultiHeadAttention (RoPE, causal) Trainium2 kernel over 8 NeuronCores.

Sharding: batch (2) x head-groups (4 heads each) -> 8 cores.
Each core computes, for its batch b and 4 heads:
  Q^T,K^T = (Wq/Wk chunk)^T @ x^T   (RoPE applied on-chip)
  S^T tiles = K^T_tile^T-contract Q^T  (d contraction), exp, causal mask
  O^T = V^T-contract P^T (k contraction), row-sums l via ones-matmul,
  O^T normalized by 1/l (broadcast matmul), then Z_partial = O @ Wo_chunk.
Host sums the 4 per-core partials of each batch.

All matmuls run in float32r (full-rate PE); storage fp32.
"""

import sys

if "/opt/trn_rl_repo" not in sys.path:
    sys.path.insert(0, "/opt/trn_rl_repo")

import numpy as np

EMBED = 2048
S = 2048
NH = 16
HD = 128
B = 2
N_CORES = 8
HPC = 4              # heads per core
CW = HPC * HD        # 512: per-core projection width
SBK = 512            # s block width
NSB = S // SBK       # 4
NEC = EMBED // 128   # 16 e-chunks
NST = S // 128       # 16 s tiles / q tiles / k tiles
ROPE_BASE = 10000.0
SCALE = 1.0 / float(np.sqrt(HD))

_CACHE = {}


def _build_program():
    import concourse.bacc as bacc
    import concourse.mybir as mybir
    import concourse.tile as tile

    f32 = mybir.dt.float32
    f32r = mybir.dt.float32r
    EXP = mybir.ActivationFunctionType.Exp

    nc = bacc.Bacc("TRN2", target_bir_lowering=False, debug=False,
                   num_devices=N_CORES)

    xt_d = nc.dram_tensor("xt", [EMBED, S], f32, kind="ExternalInput").ap()
    wq_d = nc.dram_tensor("wq", [EMBED, CW], f32, kind="ExternalInput").ap()
    wk_d = nc.dram_tensor("wk", [EMBED, CW], f32, kind="ExternalInput").ap()
    wv_d = nc.dram_tensor("wv", [EMBED, CW], f32, kind="ExternalInput").ap()
    wo_d = nc.dram_tensor("wo", [CW, EMBED], f32, kind="ExternalInput").ap()
    cos_d = nc.dram_tensor("cost", [HD, S], f32, kind="ExternalInput").ap()
    sin_d = nc.dram_tensor("sints", [HD, S], f32, kind="ExternalInput").ap()
    msk_d = nc.dram_tensor("masks", [128, 4 * SBK], f32, kind="ExternalInput").ap()
    one_d = nc.dram_tensor("ones", [128, 128], f32, kind="ExternalInput").ap()
    z_d = nc.dram_tensor("z", [S, EMBED], f32, kind="ExternalOutput").ap()

    with tile.TileContext(nc) as tc, \
         nc.allow_low_precision(reason="fp32r attention pipeline"):
        with tc.tile_pool(name="persist", bufs=1) as pp:
            qt = pp.tile([128, HPC * S], f32r, tag="qt")   # Q^T rope, per head
            kt = pp.tile([128, HPC * S], f32r, tag="kt")   # K^T rope, per head
            vt = pp.tile([128, NST * CW], f32r, tag="vt")  # V, [s-tile, 4 heads]

            # ---------------- Phase A: Q/K projections + RoPE ----------------
            with tc.tile_pool(name="wqk", bufs=1) as wp, \
                 tc.tile_pool(name="cossin", bufs=1) as cs, \
                 tc.tile_pool(name="xa", bufs=3) as xa, \
                 tc.tile_pool(name="ropetmp", bufs=2) as rp, \
                 tc.tile_pool(name="psA", bufs=8, space="PSUM") as psA:
                wq_sb = wp.tile([128, NEC * CW], f32r, tag="wq")
                wk_sb = wp.tile([128, NEC * CW], f32r, tag="wk")
                for ec in range(NEC):
                    nc.sync.dma_start(
                        wq_sb[:, ec * CW:(ec + 1) * CW],
                        wq_d[ec * 128:(ec + 1) * 128, :].bitcast(f32r))
                    nc.sync.dma_start(
                        wk_sb[:, ec * CW:(ec + 1) * CW],
                        wk_d[ec * 128:(ec + 1) * 128, :].bitcast(f32r))
                cos_sb = cs.tile([128, S], f32, tag="cos")
                sin_sb = cs.tile([128, S], f32, tag="sin")
                nc.sync.dma_start(cos_sb[:], cos_d[:])
                nc.sync.dma_start(sin_sb[:], sin_d[:])

                for sb in range(NSB):
                    qp = [psA.tile([128, SBK], f32, tag="ps", name=f"qp{sb}_{_h}") for _h in range(HPC)]
                    kp = [psA.tile([128, SBK], f32, tag="ps", name=f"kp{sb}_{_h}") for _h in range(HPC)]
                    for ec in range(NEC):
                        xtile = xa.tile([128, SBK], f32r, tag="x")
                        nc.sync.dma_start(
                            xtile[:],
                            xt_d[ec * 128:(ec + 1) * 128,
                                 sb * SBK:(sb + 1) * SBK].bitcast(f32r))
                        st, sp = (ec == 0), (ec == NEC - 1)
                        for h in range(HPC):
                            wslice = slice(ec * CW + h * HD, ec * CW + (h + 1) * HD)
                            nc.tensor.matmul(qp[h][:], lhsT=wq_sb[:, wslice],
                                             rhs=xtile[:], start=st, stop=sp)
                            nc.tensor.matmul(kp[h][:], lhsT=wk_sb[:, wslice],
                                             rhs=xtile[:], start=st, stop=sp)
                    # RoPE: out = raw*cos + swap64(raw)*sin_signed, from PSUM
                    for h in range(HPC):
                        for name, psrc, dst in (("q", qp[h], qt), ("k", kp[h], kt)):
                            t1 = rp.tile([128, SBK], f32, tag="t1")
                            t2 = rp.tile([128, SBK], f32, tag="t2")
                            ss = slice(sb * SBK, (sb + 1) * SBK)
                            nc.vector.tensor_mul(t1[:], psrc[:], cos_sb[:, ss])
                            nc.vector.tensor_mul(t2[0:64, :], psrc[64:128, :],
                                                 sin_sb[0:64, ss])
                            nc.vector.tensor_mul(t2[64:128, :], psrc[0:64, :],
                                                 sin_sb[64:128, ss])
                            ds = slice(h * S + sb * SBK, h * S + (sb + 1) * SBK)
                            nc.vector.tensor_add(dst[:, ds], t1[:], t2[:])

            # ---------------- Phase B: V projection ----------------
            with tc.tile_pool(name="wv", bufs=1) as wvp, \
                 tc.tile_pool(name="xb", bufs=3) as xb, \
                 tc.tile_pool(name="psB", bufs=4, space="PSUM") as psB:
                wv_sb = wvp.tile([128, NEC * CW], f32r, tag="wv")
                for ec in range(NEC):
                    nc.sync.dma_start(
                        wv_sb[:, ec * CW:(ec + 1) * CW],
                        wv_d[ec * 128:(ec + 1) * 128, :].bitcast(f32r))
                for sb in range(NSB):
                    vp = [psB.tile([128, CW], f32, tag="psv", name=f"vp{sb}_{_s}") for _s in range(4)]
                    for ec in range(NEC):
                        xtile = xb.tile([128, SBK], f32r, tag="xb")
                        nc.sync.dma_start(
                            xtile[:],
                            xt_d[ec * 128:(ec + 1) * 128,
                                 sb * SBK:(sb + 1) * SBK].bitcast(f32r))
                        st, sp = (ec == 0), (ec == NEC - 1)
                        for sub in range(4):
                            nc.tensor.matmul(
                                vp[sub][:],
                                lhsT=xtile[:, sub * 128:(sub + 1) * 128],
                                rhs=wv_sb[:, ec * CW:(ec + 1) * CW],
                                start=st, stop=sp)
                    for sub in range(4):
                        stile = sb * 4 + sub
                        nc.scalar.copy(vt[:, stile * CW:(stile + 1) * CW],
                                       vp[sub][:])

            # ---------------- Phase C: attention per head ----------------
            with tc.tile_pool(name="cpersist", bufs=1) as cpp:
                ot = cpp.tile([128, HPC * S], f32r, tag="ot")
                msk_sb = cpp.tile([128, 4 * SBK], f32r, tag="msk")
                nc.sync.dma_start(msk_sb[:], msk_d[:].bitcast(f32r))
                one_sb = cpp.tile([128, 128], f32r, tag="one")
                nc.sync.dma_start(one_sb[:], one_d[:].bitcast(f32r))

                with tc.tile_pool(name="pts", bufs=4) as ptp, \
                     tc.tile_pool(name="recs", bufs=2) as rcp, \
                     tc.tile_pool(name="bcs", bufs=2) as bcp, \
                     tc.tile_pool(name="psS", bufs=4, space="PSUM") as psS, \
                     tc.tile_pool(name="psAV", bufs=2, space="PSUM") as psAV, \
                     tc.tile_pool(name="psL", bufs=1, space="PSUM") as psL, \
                     tc.tile_pool(name="psBC", bufs=1, space="PSUM") as psBC:
                    for h in range(HPC):
                        hq = slice(h * S, (h + 1) * S)
                        for j in range(NSB):
                            nkt = 4 * j + 4  # causal: k tiles 0..4j+3
                            avp = psAV.tile([128, SBK], f32, tag="av")
                            lp = psL.tile([1, SBK], f32, tag="l")
                            qs = slice(h * S + j * SBK, h * S + (j + 1) * SBK)
                            for i in range(nkt):
                                sp_t = psS.tile([128, SBK], f32, tag="s")
                                ks = slice(h * S + i * 128, h * S + (i + 1) * 128)
                                nc.tensor.matmul(sp_t[:], lhsT=kt[:, ks],
                                                 rhs=qt[:, qs],
                                                 start=True, stop=True)
                                pt_sb = ptp.tile([128, SBK], f32r, tag="p")
                                nc.scalar.activation(pt_sb[:], sp_t[:], EXP,
                                                     scale=SCALE)
                                o_idx = i - 4 * j
                                if o_idx >= 0:  # diagonal-crossing tile
                                    nc.vector.tensor_mul(
                                        pt_sb[:], pt_sb[:],
                                        msk_sb[:, o_idx * SBK:(o_idx + 1) * SBK])
                                st, sp = (i == 0), (i == nkt - 1)
                                nc.tensor.matmul(
                                    avp[:],
                                    lhsT=vt[:, i * CW + h * HD:i * CW + (h + 1) * HD],
                                    rhs=pt_sb[:], start=st, stop=sp)
                                nc.tensor.matmul(lp[:], lhsT=one_sb[:, 0:1],
                                                 rhs=pt_sb[:], start=st, stop=sp)
                            rec = rcp.tile([1, SBK], f32r, tag="rec")
                            nc.vector.reciprocal(rec[:], lp[:])
                            bcps = psBC.tile([128, SBK], f32, tag="bc")
                            nc.tensor.matmul(bcps[:], lhsT=one_sb[0:1, :],
                                             rhs=rec[:], start=True, stop=True)
                            bc_sb = bcp.tile([128, SBK], f32, tag="bcs")
                            nc.scalar.copy(bc_sb[:], bcps[:])
                            nc.vector.tensor_mul(ot[:, qs], avp[:], bc_sb[:])

                # ---------------- Phase D: output projection ----------------
                with tc.tile_pool(name="wo", bufs=1) as wop, \
                     tc.tile_pool(name="zsb", bufs=4) as zp, \
                     tc.tile_pool(name="psD", bufs=4, space="PSUM") as psD:
                    wo_sb = wop.tile([128, HPC * EMBED], f32r, tag="wo")
                    for h in range(HPC):
                        nc.sync.dma_start(
                            wo_sb[:, h * EMBED:(h + 1) * EMBED],
                            wo_d[h * 128:(h + 1) * 128, :].bitcast(f32r))
                    for q_i in range(NST):
                        for eb in range(4):
                            zps = psD.tile([128, SBK], f32, tag="z")
                            for h in range(HPC):
                                nc.tensor.matmul(
                                    zps[:],
                                    lhsT=ot[:, h * S + q_i * 128:
                                            h * S + (q_i + 1) * 128],
                                    rhs=wo_sb[:, h * EMBED + eb * SBK:
                                              h * EMBED + (eb + 1) * SBK],
                                    start=(h == 0), stop=(h == HPC - 1))
                            z_sb = zp.tile([128, SBK], f32, tag="zs")
                            nc.vector.tensor_copy(z_sb[:], zps[:])
                            nc.sync.dma_start(
                                z_d[q_i * 128:(q_i + 1) * 128,
                                    eb * SBK:(eb + 1) * SBK], z_sb[:])

    nc.compile()
    return nc


def _host_tables():
    inv_freq = 1.0 / (ROPE_BASE ** (np.arange(0, HD, 2, dtype=np.float64) / HD))
    ang = np.arange(S, dtype=np.float64)[:, None] * inv_freq[None, :]  # [S, 64]
    cos = np.cos(ang)
    sin = np.sin(ang)
    cost = np.concatenate([cos, cos], axis=1).T.astype(np.float32)  # [128, S]
    sints = np.concatenate([-sin, sin], axis=1).T.astype(np.float32)
    cost = np.ascontiguousarray(cost)
    sints = np.ascontiguousarray(sints)
    kk = np.arange(128)[:, None]
    qq = np.arange(SBK)[None, :]
    masks = np.zeros((128, 4 * SBK), dtype=np.float32)
    for o in range(4):
        masks[:, o * SBK:(o + 1) * SBK] = (kk <= qq - o * 128).astype(np.float32)
    ones = np.ones((128, 128), dtype=np.float32)
    return cost, sints, masks, ones


def kernel(x, Wq, Wk, Wv, Wo):
    from concourse.bass_utils import run_bass_kernel_spmd

    x = np.asarray(x, dtype=np.float32)
    Wq = np.asarray(Wq, dtype=np.float32)
    Wk = np.asarray(Wk, dtype=np.float32)
    Wv = np.asarray(Wv, dtype=np.float32)
    Wo = np.asarray(Wo, dtype=np.float32)

    if "nc" not in _CACHE:
        _CACHE["nc"] = _build_program()
    nc = _CACHE["nc"]

    cost, sints, masks, ones = _host_tables()
    in_maps = []
    for c in range(N_CORES):
        b = c // 4
        h0 = (c % 4) * HPC * HD  # column offset of this core's 4 heads
        in_maps.append({
            "xt": np.ascontiguousarray(x[b].T),
            "wq": np.ascontiguousarray(Wq[:, h0:h0 + CW]),
            "wk": np.ascontiguousarray(Wk[:, h0:h0 + CW]),
            "wv": np.ascontiguousarray(Wv[:, h0:h0 + CW]),
            "wo": np.ascontiguousarray(Wo[h0:h0 + CW, :]),
            "cost": cost,
            "sints": sints,
            "masks": masks,
            "ones": ones,
        })

    res = run_bass_kernel_spmd(nc, in_maps, core_ids=list(range(N_CORES)))
    zs = [res.results[c]["z"] for c in range(N_CORES)]
    out = np.empty((B, S, EMBED), dtype=np.float32)
    out[0] = zs[0] + zs[1] + zs[2] + zs[3]
    out[1] = zs[4] + zs[5] + zs[6] + zs[7]
    return out
